# revision 33
# baseline (speedup 1.0000x reference)
"""GATv2 message-passing kernel for 8 Trainium2 NeuronCores (Bass/Tile).

Strategy
--------
Nodes are partitioned into 8 contiguous ranges (one per core). Every edge is
assigned to the core that owns its *receiver*, so each core computes the
complete softmax + weighted aggregation for its own nodes with no collectives.

Host-side preprocessing (index-driven data movement only, no FLOPs):
  * group edges by (core, receiver-tile-of-128), pad each tile's edge list to
    a common chunk count, and lay the edge features out transposed
    ([feat, edge]) for direct use as matmul operands;
  * pre-gather raw sender node features per edge (nodes[senders]) so the
    device reads a sequential stream instead of doing a random-access gather;
  * both big streams are stored bf16 (halves HBM traffic; rel-err stays
    ~8e-3, well under the 2e-2 gate).

Device pipeline per receiver tile (128 nodes), per group of <=4 edge chunks
(chunk = 128 edges), with zT meaning "transposed [dim, edge] layout":
  bc    = ones.T @ rr                           (broadcast recv row, PSUM)
  ohne  = (bc == iota_p)                        (DVE, [node, edge] one-hot)
  zT    = We.T@edgesT + Ws.T@sentT + hr_tile@ohne   (PSUM accumulate)
  x     = PRelu(zT, 0.01) -> bf16               (ACT)
  lg2   = x_chunk.T @ ablk                      (logits in [edge, head], PSUM)
  ex2   = Exp(lg2)                              (ACT, [edge, 4, 8])
  spj   = sentT_chunk.T @ Ws                    ([edge, dim] sent projection)
  rhs4  = [spj * broadcast(ex2) | ex2]          ([edge, 136] bf16)
  ohen  = (iota == rloc_chunk)                  (DVE, [edge, node] one-hot)
  acc  += ohen.T @ rhs4                         (scatter matmul, [node, 136])
Epilogue per tile: out = acc[:, :128] / (acc[:, 128:136] + eps), DMA out.
"""
import os
import sys

sys.path.insert(0, "/opt/trn_rl_repo")

import numpy as np
import ml_dtypes
import concourse.bass as bass
import concourse.bacc as bacc
import concourse.mybir as mybir
import concourse.tile as tile

F32 = mybir.dt.float32
BF16 = mybir.dt.bfloat16
NPBF = ml_dtypes.bfloat16

NCORES = 8
P = 128
HEADS = 8
HDIM = 16
GROUP = 4

LAST_EXEC_NS = None
LAST_PROFILE = None
LAST_BENCH_NS = None


def kernel(nodes, edges, senders, receivers, Ws_k, Ws_b, Wr_k, Wr_b, We_k, We_b, a):
    global LAST_EXEC_NS, LAST_PROFILE

    nodes = np.asarray(nodes, dtype=np.float32)
    edges = np.asarray(edges, dtype=np.float32)
    senders = np.asarray(senders, dtype=np.int32)
    receivers = np.asarray(receivers, dtype=np.int32)
    Ws_k = np.asarray(Ws_k, dtype=np.float32)
    Ws_b = np.asarray(Ws_b, dtype=np.float32)
    Wr_k = np.asarray(Wr_k, dtype=np.float32)
    Wr_b = np.asarray(Wr_b, dtype=np.float32)
    We_k = np.asarray(We_k, dtype=np.float32)
    We_b = np.asarray(We_b, dtype=np.float32)
    a = np.asarray(a, dtype=np.float32)

    N, D = nodes.shape
    E = edges.shape[0]
    assert D == 128 and Ws_k.shape == (128, 128)
    assert N % NCORES == 0
    NLOC = N // NCORES
    NTILES = (NLOC + P - 1) // P
    NLOC_PAD = NTILES * P

    # ---------------- host-side sharding / layout ----------------
    core = receivers // NLOC
    rloc_in_core = receivers - core * NLOC

    # Balance receiver tiles per core: LPT-assign local nodes to NTILES bins
    # (capacity 128) by in-degree so the max per-tile edge count (=> T_max,
    # the padded chunk count every tile pays) is near the mean.
    import heapq
    tile_of = np.zeros((NCORES, NLOC), np.int32)
    pos_of = np.zeros((NCORES, NLOC), np.int32)
    l2g = np.full((NCORES, NLOC_PAD), -1, np.int64)   # padded slot -> local id
    for ci in range(NCORES):
        deg = np.bincount(rloc_in_core[core == ci], minlength=NLOC)
        order_n = np.argsort(-deg, kind="stable")
        heap = [(0, t) for t in range(NTILES)]
        heapq.heapify(heap)
        counts = np.zeros(NTILES, np.int32)
        for lid in order_n:
            load, t = heapq.heappop(heap)
            tile_of[ci, lid] = t
            pos_of[ci, lid] = counts[t]
            l2g[ci, t * P + counts[t]] = lid
            counts[t] += 1
            if counts[t] < P:
                heapq.heappush(heap, (load + int(deg[lid]), t))

    tl = tile_of[core, rloc_in_core]            # receiver tile within core
    recv_local = pos_of[core, rloc_in_core].astype(np.float32)
    gt = core * NTILES + tl                     # global (core,tile) bucket

    order = np.argsort(gt, kind="stable")
    gt_sorted = gt[order]
    cnt = np.bincount(gt_sorted, minlength=NCORES * NTILES)
    T_max = max(1, int(-(-cnt.max() // P)))     # chunks per tile, all cores
    NCHUNK = NTILES * T_max
    E_pad = NCHUNK * P

    # slot of each (sorted) edge inside its core's stream
    starts = np.zeros(NCORES * NTILES + 1, dtype=np.int64)
    np.cumsum(cnt, out=starts[1:])
    rank = np.arange(E, dtype=np.int64) - starts[gt_sorted]
    slot = (gt_sorted % NTILES) * (T_max * P) + rank

    edges_bf = edges.astype(NPBF)
    nodes_bf = nodes.astype(NPBF)
    sent_bf = nodes_bf[senders]                 # [E, 128] host gather

    EDG = np.zeros((NCORES, P, E_pad), dtype=NPBF)
    SRT = np.zeros((NCORES, P, E_pad), dtype=NPBF)
    RROW = np.full((NCORES, 1, E_pad), -1.0, dtype=NPBF)
    RLOC = np.full((NCORES, P, NCHUNK), -1.0, dtype=np.float32)
    for ci in range(NCORES):
        m = gt_sorted // NTILES == ci
        sel = order[m]
        sl = slot[m]
        EDG[ci][:, sl] = edges_bf[sel].T
        SRT[ci][:, sl] = sent_bf[sel].T
        RROW[ci][0, sl] = recv_local[sel].astype(NPBF)
        RLOC[ci][sl % P, sl // P] = recv_local[sel]

    # local (per-core) transposed node features for the hr projection,
    # laid out in the permuted (tile-balanced) node order
    NLT = np.zeros((NCORES, P, NLOC_PAD), dtype=NPBF)
    for ci in range(NCORES):
        valid = l2g[ci] >= 0
        NLT[ci][:, valid] = nodes_bf[ci * NLOC + l2g[ci][valid]].T

    # block-diagonal attention vector [128, 8]
    ablk = np.zeros((P, HEADS), dtype=np.float32)
    for h in range(HEADS):
        ablk[h * HDIM:(h + 1) * HDIM, h] = a[h]

    b_all = Ws_b + Wr_b + We_b
    add_bias = bool(np.any(b_all != 0.0))

    # bf16 const block (cols):
    #   0:128    We
    #   128:256  Ws
    #   256:384  Wr
    #   384:392  ablk
    #   392:904  ones (row 0 used as [1, W] matmul lhsT/rhs)
    #   904:1032 bias row ([1, 128] b_all in row 0)
    #   1032:1160 iota rows (iota[p, j] = j)
    CW16 = 1160
    C16 = np.zeros((P, CW16), dtype=np.float32)
    C16[:, 0:128] = We_k
    C16[:, 128:256] = Ws_k
    C16[:, 256:384] = Wr_k
    C16[:, 384:392] = ablk
    C16[:, 392:904] = 1.0
    C16[0, 904:1032] = b_all
    C16[:, 1032:1160] = np.arange(P, dtype=np.float32)[None, :]
    C16 = C16.astype(NPBF)

    # f32 const block (cols):
    #   0:128  iota rows   (iota[p, j] = j)
    #   128    iotaC       (iotaC[p] = p)
    CW32 = 129
    C32 = np.zeros((P, CW32), dtype=np.float32)
    C32[:, 0:128] = np.arange(P, dtype=np.float32)[None, :]
    C32[:, 128] = np.arange(P, dtype=np.float32)

    # ---------------- build the bass program ----------------
    GROUPS = []
    g0 = 0
    while g0 < T_max:
        GROUPS.append((g0, min(GROUP, T_max - g0)))
        g0 += GROUP

    nc = bacc.Bacc("TRN2", target_bir_lowering=False, debug=False)

    d_edg = nc.declare_dram_parameter("EDG", [P, E_pad], BF16, isOutput=False)
    d_srt = nc.declare_dram_parameter("SRT", [P, E_pad], BF16, isOutput=False)
    d_rrow = nc.declare_dram_parameter("RROW", [1, E_pad], BF16, isOutput=False)
    d_rloc = nc.declare_dram_parameter("RLOC", [P, NCHUNK], F32, isOutput=False)
    d_nlt = nc.declare_dram_parameter("NLT", [P, NLOC_PAD], BF16, isOutput=False)
    d_cb16 = nc.declare_dram_parameter("C16", [P, CW16], BF16, isOutput=False)
    d_cb32 = nc.declare_dram_parameter("C32", [P, CW32], F32, isOutput=False)
    d_nrep = nc.declare_dram_parameter("NREP", [1, 1], mybir.dt.int32,
                                       isOutput=False)
    d_out = nc.declare_dram_parameter("OUT", [NLOC_PAD, P], F32, isOutput=True)

    PRELU = mybir.ActivationFunctionType.Prelu
    EXP = mybir.ActivationFunctionType.Exp
    COPY = mybir.ActivationFunctionType.Copy
    EQ = mybir.AluOpType.is_equal
    MUL = mybir.AluOpType.mult
    ADD = mybir.AluOpType.add

    with tile.TileContext(nc) as tc:
        with (
            tc.tile_pool(name="cst", bufs=1) as cpool,
            tc.tile_pool(name="sbio", bufs=3) as sbio,
            tc.tile_pool(name="sb", bufs=2) as sb,
            tc.tile_pool(name="ps_zt", bufs=2, space="PSUM") as ps_zt,
            tc.tile_pool(name="ps_spj", bufs=2, space="PSUM") as ps_spj,
            tc.tile_pool(name="ps_acc", bufs=2, space="PSUM") as ps_acc,
            tc.tile_pool(name="ps_lg", bufs=2, space="PSUM") as ps_lg,
        ):
            cb = cpool.tile([P, CW16], BF16)
            nc.sync.dma_start(out=cb[:], in_=d_cb16[:])
            c32 = cpool.tile([P, CW32], F32)
            nc.sync.dma_start(out=c32[:], in_=d_cb32[:])
            rloc = cpool.tile([P, NCHUNK], F32)
            nc.sync.dma_start(out=rloc[:], in_=d_rloc[:])
            nlt = cpool.tile([P, NLOC_PAD], BF16)
            nc.sync.dma_start(out=nlt[:], in_=d_nlt[:])
            hr_sb = cpool.tile([P, NLOC_PAD], BF16)

            c_We = cb[:, 0:128]
            c_Ws = cb[:, 128:256]
            c_Wr = cb[:, 256:384]
            c_ablk = cb[:, 384:392]
            c_ones = cb[0:1, 392:904]
            c_brow = cb[0:1, 904:1032]
            c_iota16 = cb[:, 1032:1160]
            c_iotaC = c32[:, 128:129]

            use_inloop = os.environ.get("GAT_INLOOP", "0") == "1"
            rep_ctx = None
            if use_inloop:
                nrep_sb = cpool.tile([1, 1], mybir.dt.int32)
                nc.sync.dma_start(out=nrep_sb[:], in_=d_nrep[:])
                n_rep = nc.values_load(nrep_sb[0:1, 0:1], min_val=1,
                                       max_val=4096)
                rep_ctx = tc.For_i(0, n_rep, 1)
                rep_ctx.__enter__()

            # ---- prologue: hr projection for local nodes ----
            for t in range(NTILES):
                pp = ps_lg.tile([P, 128], F32, tag="lg")
                nc.tensor.matmul(
                    out=pp[:], lhsT=nlt[:, t * P:(t + 1) * P],
                    rhs=c_Wr, start=True, stop=not add_bias,
                )
                if add_bias:
                    nc.tensor.matmul(
                        out=pp[:], lhsT=cb[0:1, 392:393], rhs=c_brow,
                        start=False, stop=True,
                    )
                dst = hr_sb[:, t * P:(t + 1) * P]
                if t % 2 == 0:
                    nc.scalar.activation(dst, pp[:], COPY)
                else:
                    nc.vector.tensor_copy(out=dst, in_=pp[:])

            # ---- main loop over receiver tiles ----
            for t in range(NTILES):
                co = t * T_max * P
                edg = sbio.tile([P, T_max * P], BF16, tag="edg")
                nc.sync.dma_start(out=edg[:], in_=d_edg[:, co:co + T_max * P])
                srt = sbio.tile([P, T_max * P], BF16, tag="srt")
                nc.sync.dma_start(out=srt[:], in_=d_srt[:, co:co + T_max * P])
                # receiver row replicated across partitions by the DMA
                rrb = sbio.tile([P, T_max * P], BF16, tag="rrb")
                nc.sync.dma_start(
                    out=rrb[:],
                    in_=d_rrow[0:1, co:co + T_max * P].to_broadcast(
                        [P, T_max * P]),
                )
                # whole-tile [node, edge] one-hot in one 4x-mode DVE op
                ohne = sb.tile([P, T_max * P], BF16, tag="ohne")
                nc.vector.tensor_scalar(out=ohne[:], in0=rrb[:],
                                        scalar1=c_iotaC, scalar2=None,
                                        op0=EQ)

                acc = ps_acc.tile([P, 136], F32, tag="acc")
                hr_t = hr_sb[:, t * P:(t + 1) * P]
                n_sc = 0

                for gi, (gc0, ncg) in enumerate(GROUPS):
                    W = ncg * P
                    csl = slice(gc0 * P, gc0 * P + W)

                    zT = ps_zt.tile([P, 512], F32, tag="zT")
                    nc.tensor.matmul(out=zT[:, 0:W], lhsT=c_We, rhs=edg[:, csl],
                                     start=True, stop=False)
                    nc.tensor.matmul(out=zT[:, 0:W], lhsT=c_Ws, rhs=srt[:, csl],
                                     start=False, stop=False)
                    nc.tensor.matmul(out=zT[:, 0:W], lhsT=hr_t,
                                     rhs=ohne[:, csl],
                                     start=False, stop=not add_bias)
                    if add_bias:
                        nc.tensor.matmul(out=zT[:, 0:W], lhsT=c_brow,
                                         rhs=c_ones[:, 0:W],
                                         start=False, stop=True)

                    x = sb.tile([P, GROUP * P], BF16, tag="x")
                    nc.scalar.activation(x[:, 0:W], zT[:, 0:W], PRELU,
                                         alpha=0.01)

                    # logits directly in [edge, head] layout, per chunk
                    lg2 = ps_lg.tile([P, GROUP, 8], F32, tag="lg")
                    for c in range(ncg):
                        nc.tensor.matmul(
                            out=lg2[:, c, :],
                            lhsT=x[:, c * P:(c + 1) * P],
                            rhs=c_ablk, start=True, stop=True,
                        )
                    ex2 = sb.tile([P, GROUP, 8], F32, tag="ex2")
                    nc.scalar.activation(ex2[:, 0:ncg, :], lg2[:, 0:ncg, :],
                                         EXP)

                    # sent projection in [edge, dim] layout, per chunk
                    spj = ps_spj.tile([P, GROUP, 128], F32, tag="spj")
                    for c in range(ncg):
                        nc.tensor.matmul(
                            out=spj[:, c, :],
                            lhsT=srt[:, (gc0 + c) * P:(gc0 + c + 1) * P],
                            rhs=c_Ws, start=True, stop=True,
                        )

                    # scatter rhs: [weighted msg (128) | ex (8)] per chunk
                    rhs4 = sb.tile([P, GROUP, 136], BF16, tag="rhs4")
                    nc.scalar.activation(rhs4[:, 0:ncg, 128:136],
                                         ex2[:, 0:ncg, :], COPY)
                    nc.vector.tensor_tensor(
                        out=rhs4[:, 0:ncg, 0:128].rearrange(
                            "p c (h j) -> p c h j", h=8),
                        in0=spj[:, 0:ncg, :].rearrange(
                            "p c (h j) -> p c h j", h=8),
                        in1=ex2[:, 0:ncg, :].to_broadcast([P, ncg, 8, 16]),
                        op=MUL,
                    )

                    for c in range(ncg):
                        ohen = sb.tile([P, P], BF16, tag="ohen")
                        nc.vector.tensor_scalar(
                            out=ohen[:], in0=c_iota16,
                            scalar1=rloc[:, t * T_max + gc0 + c:
                                         t * T_max + gc0 + c + 1],
                            scalar2=None, op0=EQ,
                        )
                        n_sc += 1
                        nc.tensor.matmul(
                            out=acc[:],
                            lhsT=ohen[:],
                            rhs=rhs4[:, c, :],
                            start=(n_sc == 1), stop=(n_sc == T_max),
                        )

                # ---- epilogue ----
                dsb = sb.tile([P, 8], F32, tag="dsb")
                nc.vector.tensor_scalar(out=dsb[:], in0=acc[:, 128:136],
                                        scalar1=1e-30, scalar2=None, op0=ADD)
                rec = sb.tile([P, 8], F32, tag="rec")
                nc.vector.reciprocal(out=rec[:], in_=dsb[:])
                ot = sb.tile([P, P], F32, tag="ot")
                nc.vector.tensor_tensor(
                    out=ot[:].rearrange("p (h j) -> p h j", h=8),
                    in0=acc[:, 0:128].rearrange("p (h j) -> p h j", h=8),
                    in1=rec[:].to_broadcast([P, 8, 16]),
                    op=MUL,
                )
                nc.sync.dma_start(out=d_out[t * P:(t + 1) * P, :], in_=ot[:])

            if rep_ctx is not None:
                rep_ctx.__exit__(None, None, None)

    nc.compile()

    in_maps = [
        dict(EDG=EDG[ci], SRT=SRT[ci], RROW=RROW[ci], RLOC=RLOC[ci],
             NLT=NLT[ci], C16=C16, C32=C32,
             NREP=np.array([[int(os.environ.get("GAT_NREP", "1"))]],
                           dtype=np.int32))
        for ci in range(NCORES)
    ]

    def unpermute(ci, rows):
        out_c = np.zeros((NLOC, P), np.float32)
        valid = l2g[ci] >= 0
        out_c[l2g[ci][valid]] = rows[valid]
        return out_c

    if os.environ.get("GAT_SIM", "0") == "1":
        out0 = _run_sim(nc, in_maps[0])
        out = np.zeros((N, P), np.float32)
        out[:NLOC] = unpermute(0, out0)
        return out

    bench_iters = int(os.environ.get("GAT_BENCH", "10"))
    results = _run_pjrt(nc, in_maps, NCORES, bench_iters)
    out = np.concatenate(
        [unpermute(ci, results[ci]["OUT"]) for ci in range(NCORES)], axis=0
    )
    return out.astype(np.float32)


def _run_sim(nc, in_map):
    """Run core 0 through CoreSim: numerics check + cost-model timing."""
    from concourse.bass_interp import CoreSim, InstructionExecutor, Direction
    import concourse.mybir as mb

    class _Exec(InstructionExecutor):
        # CoreSim has no Prelu implementation; emulate it here (dev-only).
        def visit_InstActivation(self, instruction, *, reg_snapshot=None):
            if instruction.func != mb.ActivationFunctionType.Prelu:
                return super().visit_InstActivation(
                    instruction, reg_snapshot=reg_snapshot)
            alpha = instruction.ins[3]
            alpha = alpha.value if isinstance(alpha, mb.ImmediateValue) else 0.0
            iv = self.view_ap(instruction.ins[0], Direction.READ, instruction,
                              reg_snapshot=reg_snapshot).astype(np.float32)
            acted = np.where(iv > 0, iv, np.float32(alpha) * iv)
            ov = self.view_ap(instruction.outs[0], Direction.WRITE,
                              instruction, reg_snapshot=reg_snapshot)
            ov[:] = acted.astype(ov.dtype)

    sim = CoreSim(nc, publish_trace=False, executor_cls=_Exec)
    for name, arr in in_map.items():
        view = sim.tensor(name)
        view[:] = np.asarray(arr, dtype=view.dtype)
    sim.simulate()
    print(f"SIM time: {sim.time} ns")
    return np.array(sim.tensor("OUT"), dtype=np.float32)


def _run_pjrt(nc, in_maps, n_cores, bench_iters=0):
    """Execute the compiled module on the PJRT/axon devices; optionally
    re-run with pre-staged device inputs to measure steady-state latency."""
    global LAST_EXEC_NS, LAST_BENCH_NS
    import time as _time
    import jax
    from jax.sharding import Mesh, PartitionSpec, NamedSharding
    from jax.experimental.shard_map import shard_map
    import concourse.mybir as _mb
    from concourse import bass2jax as _b2j

    _b2j.install_neuronx_cc_hook()

    in_names, out_names, out_avals, zero_outs = [], [], [], []
    for alloc in nc.m.functions[0].allocations:
        if not isinstance(_mb.MemoryLocationSet, type) or not isinstance(alloc, _mb.MemoryLocationSet):
            continue
        name = alloc.memorylocations[0].name
        if alloc.kind == "ExternalInput":
            if nc.partition_id_tensor is None or name != nc.partition_id_tensor.name:
                in_names.append(name)
        elif alloc.kind == "ExternalOutput":
            out_names.append(name)
            shape = tuple(alloc.tensor_shape)
            dtype = _mb.dt.np(alloc.dtype)
            out_avals.append(jax.core.ShapedArray(shape, dtype))
            zero_outs.append(np.zeros(shape, dtype))
    n_params = len(in_names)
    n_outs = len(out_avals)
    in_names = in_names + out_names

    part_name = nc.partition_id_tensor.name if nc.partition_id_tensor else None
    if part_name is not None:
        in_names.append(part_name)

    def _body(*args):
        operands = list(args)
        if part_name is not None:
            operands.append(_b2j.partition_id_tensor())
        outs = _b2j._bass_exec_p.bind(
            *operands,
            out_avals=tuple(out_avals),
            in_names=tuple(in_names),
            out_names=tuple(out_names),
            lowering_input_output_aliases=(),
            sim_require_finite=True,
            sim_require_nnan=True,
            nc=nc,
        )
        return tuple(outs)

    devices = jax.devices()[:n_cores]
    mesh = Mesh(np.asarray(devices), ("core",))
    in_specs = (PartitionSpec("core"),) * (n_params + n_outs)
    out_specs = (PartitionSpec("core"),) * n_outs
    fn = jax.jit(
        shard_map(_body, mesh=mesh, in_specs=in_specs,
                  out_specs=out_specs, check_rep=False),
        keep_unused=True,
    )
    sh = NamedSharding(mesh, PartitionSpec("core"))
    concat_in = [
        jax.device_put(
            np.concatenate([np.asarray(in_maps[c][in_names[i]])
                            for c in range(n_cores)], axis=0), sh)
        for i in range(n_params)
    ]
    concat_zeros = [
        jax.device_put(np.zeros((n_cores * z.shape[0], *z.shape[1:]), z.dtype), sh)
        for z in zero_outs
    ]
    out_arrs = fn(*concat_in, *concat_zeros)
    jax.block_until_ready(out_arrs)

    if bench_iters > 0:
        use_inloop = os.environ.get("GAT_INLOOP", "0") == "1"
        if use_inloop:
            # Steady-state per-pass device time: the NEFF contains a dynamic
            # repeat loop over the whole kernel body (count = NREP input).
            # Time one execution at NREP=k1 and one at NREP=k2 and take the
            # slope: per-pass time measured entirely on-device, free of the
            # host<->terminal sync latency (~90ms) and per-dispatch overhead.
            nrep_idx = in_names.index("NREP")
            k1, k2 = 2, 2 + bench_iters

            def run_point(k):
                arr = jax.device_put(
                    np.full((n_cores, 1), k, np.int32), sh)
                inputs = list(concat_in)
                inputs[nrep_idx] = arr
                t0 = _time.perf_counter()
                o = fn(*inputs, *concat_zeros)
                jax.block_until_ready(o)
                return _time.perf_counter() - t0
        else:
            # Amortized per-execution time: queue N executions back-to-back,
            # block once; the slope between two batch sizes removes the fixed
            # host<->terminal sync latency (~90ms on axon).
            k1, k2 = bench_iters, 3 * bench_iters

            def run_point(n):
                t0 = _time.perf_counter()
                outs = [fn(*concat_in, *concat_zeros) for _ in range(n)]
                jax.block_until_ready(outs)
                return _time.perf_counter() - t0

        run_point(k1)  # warm
        run_point(k2)
        best = None
        for _ in range(4):
            t1 = run_point(k1)
            t2 = run_point(k2)
            slope = (t2 - t1) / (k2 - k1)
            if os.environ.get("GAT_VERBOSE", "0") == "1":
                print(f"pt {k1}: {t1*1e3:.2f} ms, pt {k2}: {t2*1e3:.2f} "
                      f"ms, slope {slope*1e3:.3f} ms/exec")
            if best is None or slope < best:
                best = slope
        LAST_BENCH_NS = int(best * 1e9)
        LAST_EXEC_NS = LAST_BENCH_NS

    np_outs = [np.asarray(a) for a in out_arrs]
    return [
        {name: np_outs[i].reshape(n_cores, *out_avals[i].shape)[c]
         for i, name in enumerate(out_names)}
        for c in range(n_cores)
    ]


# revision 44
# speedup vs baseline: 2.8670x; 2.8670x over previous
"""GATv2 message-passing kernel for 8 Trainium2 NeuronCores (Bass/Tile).

Strategy
--------
Nodes are partitioned into 8 contiguous ranges (one per core). Every edge is
assigned to the core that owns its *receiver*, so each core computes the
complete softmax + weighted aggregation for its own nodes with no collectives.

Host-side preprocessing (index-driven data movement only, no FLOPs):
  * group edges by (core, receiver-tile-of-128), pad each tile's edge list to
    a common chunk count, and lay the edge features out transposed
    ([feat, edge]) for direct use as matmul operands;
  * pre-gather raw sender node features per edge (nodes[senders]) so the
    device reads a sequential stream instead of doing a random-access gather;
  * both big streams are stored bf16 (halves HBM traffic; rel-err stays
    ~8e-3, well under the 2e-2 gate).

Device pipeline per receiver tile (128 nodes), per group of <=4 edge chunks
(chunk = 128 edges), with zT meaning "transposed [dim, edge] layout":
  bc    = ones.T @ rr                           (broadcast recv row, PSUM)
  ohne  = (bc == iota_p)                        (DVE, [node, edge] one-hot)
  zT    = We.T@edgesT + Ws.T@sentT + hr_tile@ohne   (PSUM accumulate)
  x     = PRelu(zT, 0.01) -> bf16               (ACT)
  lg2   = x_chunk.T @ ablk                      (logits in [edge, head], PSUM)
  ex2   = Exp(lg2)                              (ACT, [edge, 4, 8])
  spj   = sentT_chunk.T @ Ws                    ([edge, dim] sent projection)
  rhs4  = [spj * broadcast(ex2) | ex2]          ([edge, 136] bf16)
  ohen  = (iota == rloc_chunk)                  (DVE, [edge, node] one-hot)
  acc  += ohen.T @ rhs4                         (scatter matmul, [node, 136])
Epilogue per tile: out = acc[:, :128] / (acc[:, 128:136] + eps), DMA out.
"""
import os
import sys

sys.path.insert(0, "/opt/trn_rl_repo")

import numpy as np
import ml_dtypes
import concourse.bass as bass
import concourse.bacc as bacc
import concourse.mybir as mybir
import concourse.tile as tile

F32 = mybir.dt.float32
BF16 = mybir.dt.bfloat16
NPBF = ml_dtypes.bfloat16

NCORES = 8
P = 128
HEADS = 8
HDIM = 16
GROUP = 4

LAST_EXEC_NS = None
LAST_PROFILE = None
LAST_BENCH_NS = None


def kernel(nodes, edges, senders, receivers, Ws_k, Ws_b, Wr_k, Wr_b, We_k, We_b, a):
    global LAST_EXEC_NS, LAST_PROFILE

    nodes = np.asarray(nodes, dtype=np.float32)
    edges = np.asarray(edges, dtype=np.float32)
    senders = np.asarray(senders, dtype=np.int32)
    receivers = np.asarray(receivers, dtype=np.int32)
    Ws_k = np.asarray(Ws_k, dtype=np.float32)
    Ws_b = np.asarray(Ws_b, dtype=np.float32)
    Wr_k = np.asarray(Wr_k, dtype=np.float32)
    Wr_b = np.asarray(Wr_b, dtype=np.float32)
    We_k = np.asarray(We_k, dtype=np.float32)
    We_b = np.asarray(We_b, dtype=np.float32)
    a = np.asarray(a, dtype=np.float32)

    N, D = nodes.shape
    E = edges.shape[0]
    assert D == 128 and Ws_k.shape == (128, 128)
    assert N % NCORES == 0
    NLOC = N // NCORES
    NTILES = (NLOC + P - 1) // P
    NLOC_PAD = NTILES * P

    # ---------------- host-side sharding / layout ----------------
    core = receivers // NLOC
    rloc_in_core = receivers - core * NLOC

    # Balance receiver tiles per core: LPT-assign local nodes to NTILES bins
    # (capacity 128) by in-degree so the max per-tile edge count (=> T_max,
    # the padded chunk count every tile pays) is near the mean.
    import heapq
    tile_of = np.zeros((NCORES, NLOC), np.int32)
    pos_of = np.zeros((NCORES, NLOC), np.int32)
    l2g = np.full((NCORES, NLOC_PAD), -1, np.int64)   # padded slot -> local id
    for ci in range(NCORES):
        deg = np.bincount(rloc_in_core[core == ci], minlength=NLOC)
        order_n = np.argsort(-deg, kind="stable")
        heap = [(0, t) for t in range(NTILES)]
        heapq.heapify(heap)
        counts = np.zeros(NTILES, np.int32)
        for lid in order_n:
            load, t = heapq.heappop(heap)
            tile_of[ci, lid] = t
            pos_of[ci, lid] = counts[t]
            l2g[ci, t * P + counts[t]] = lid
            counts[t] += 1
            if counts[t] < P:
                heapq.heappush(heap, (load + int(deg[lid]), t))

    tl = tile_of[core, rloc_in_core]            # receiver tile within core
    recv_local = pos_of[core, rloc_in_core].astype(np.float32)
    gt = core * NTILES + tl                     # global (core,tile) bucket

    order = np.argsort(gt, kind="stable")
    gt_sorted = gt[order]
    cnt = np.bincount(gt_sorted, minlength=NCORES * NTILES)
    T_max = max(1, int(-(-cnt.max() // P)))     # chunks per tile, all cores
    NCHUNK = NTILES * T_max
    E_pad = NCHUNK * P

    # slot of each (sorted) edge inside its core's stream
    starts = np.zeros(NCORES * NTILES + 1, dtype=np.int64)
    np.cumsum(cnt, out=starts[1:])
    rank = np.arange(E, dtype=np.int64) - starts[gt_sorted]
    slot = (gt_sorted % NTILES) * (T_max * P) + rank

    edges_bf = edges.astype(NPBF)
    nodes_bf = nodes.astype(NPBF)
    sent_bf = nodes_bf[senders]                 # [E, 128] host gather

    EDG = np.zeros((NCORES, P, E_pad), dtype=NPBF)
    SRT = np.zeros((NCORES, P, E_pad), dtype=NPBF)
    RROW = np.full((NCORES, 1, E_pad), -1.0, dtype=NPBF)
    RLOC = np.full((NCORES, P, NCHUNK), -1.0, dtype=np.float32)
    for ci in range(NCORES):
        m = gt_sorted // NTILES == ci
        sel = order[m]
        sl = slot[m]
        EDG[ci][:, sl] = edges_bf[sel].T
        SRT[ci][:, sl] = sent_bf[sel].T
        RROW[ci][0, sl] = recv_local[sel].astype(NPBF)
        RLOC[ci][sl % P, sl // P] = recv_local[sel]

    # local (per-core) transposed node features for the hr projection,
    # laid out in the permuted (tile-balanced) node order
    NLT = np.zeros((NCORES, P, NLOC_PAD), dtype=NPBF)
    for ci in range(NCORES):
        valid = l2g[ci] >= 0
        NLT[ci][:, valid] = nodes_bf[ci * NLOC + l2g[ci][valid]].T

    # block-diagonal attention vector [128, 8]
    ablk = np.zeros((P, HEADS), dtype=np.float32)
    for h in range(HEADS):
        ablk[h * HDIM:(h + 1) * HDIM, h] = a[h]

    b_all = Ws_b + Wr_b + We_b
    add_bias = bool(np.any(b_all != 0.0))

    # bf16 const block (cols):
    #   0:128    We
    #   128:256  Ws
    #   256:384  Wr
    #   384:392  ablk
    #   392:904  ones (row 0 used as [1, W] matmul lhsT/rhs)
    #   904:1032 bias row ([1, 128] b_all in row 0)
    #   1032:1160 iota rows (iota[p, j] = j)
    CW16 = 1160
    C16 = np.zeros((P, CW16), dtype=np.float32)
    C16[:, 0:128] = We_k
    C16[:, 128:256] = Ws_k
    C16[:, 256:384] = Wr_k
    C16[:, 384:392] = ablk
    C16[:, 392:904] = 1.0
    C16[0, 904:1032] = b_all
    C16[:, 1032:1160] = np.arange(P, dtype=np.float32)[None, :]
    C16 = C16.astype(NPBF)

    # f32 const block (cols):
    #   0:128  iota rows   (iota[p, j] = j)
    #   128    iotaC       (iotaC[p] = p)
    CW32 = 129
    C32 = np.zeros((P, CW32), dtype=np.float32)
    C32[:, 0:128] = np.arange(P, dtype=np.float32)[None, :]
    C32[:, 128] = np.arange(P, dtype=np.float32)

    # ---------------- build the bass program ----------------
    GROUPS = []
    g0 = 0
    while g0 < T_max:
        GROUPS.append((g0, min(GROUP, T_max - g0)))
        g0 += GROUP

    nc = bacc.Bacc("TRN2", target_bir_lowering=False, debug=False)

    d_edg = nc.declare_dram_parameter("EDG", [P, E_pad], BF16, isOutput=False)
    d_srt = nc.declare_dram_parameter("SRT", [P, E_pad], BF16, isOutput=False)
    d_rrow = nc.declare_dram_parameter("RROW", [1, E_pad], BF16, isOutput=False)
    d_rloc = nc.declare_dram_parameter("RLOC", [P, NCHUNK], F32, isOutput=False)
    d_nlt = nc.declare_dram_parameter("NLT", [P, NLOC_PAD], BF16, isOutput=False)
    d_cb16 = nc.declare_dram_parameter("C16", [P, CW16], BF16, isOutput=False)
    d_cb32 = nc.declare_dram_parameter("C32", [P, CW32], F32, isOutput=False)
    d_nrep = nc.declare_dram_parameter("NREP", [1, 1], mybir.dt.int32,
                                       isOutput=False)
    d_out = nc.declare_dram_parameter("OUT", [NLOC_PAD, P], F32, isOutput=True)

    PRELU = mybir.ActivationFunctionType.Prelu
    EXP = mybir.ActivationFunctionType.Exp
    COPY = mybir.ActivationFunctionType.Copy
    EQ = mybir.AluOpType.is_equal
    MUL = mybir.AluOpType.mult
    ADD = mybir.AluOpType.add

    with tile.TileContext(nc) as tc:
        use_bcast = os.environ.get("GAT_BCAST", "1") == "1"
        with (
            tc.tile_pool(name="cst", bufs=1) as cpool,
            tc.tile_pool(name="sbio", bufs=3) as sbio,
            tc.tile_pool(name="sb", bufs=3) as sb,
            tc.tile_pool(name="ps_zt", bufs=2, space="PSUM") as ps_zt,
            tc.tile_pool(name="ps_spj", bufs=2, space="PSUM") as ps_spj,
            tc.tile_pool(name="ps_acc", bufs=2, space="PSUM") as ps_acc,
            tc.tile_pool(name="ps_lg", bufs=2 if use_bcast else 1,
                         space="PSUM") as ps_lg,
            tc.tile_pool(name="ps_bc", bufs=1, space="PSUM") as ps_bc,
        ):
            cb = cpool.tile([P, CW16], BF16)
            nc.sync.dma_start(out=cb[:], in_=d_cb16[:])
            c32 = cpool.tile([P, CW32], F32)
            nc.sync.dma_start(out=c32[:], in_=d_cb32[:])
            rloc = cpool.tile([P, NCHUNK], F32)
            nc.sync.dma_start(out=rloc[:], in_=d_rloc[:])
            nlt = cpool.tile([P, NLOC_PAD], BF16)
            nc.sync.dma_start(out=nlt[:], in_=d_nlt[:])
            hr_sb = cpool.tile([P, NLOC_PAD], BF16)

            c_We = cb[:, 0:128]
            c_Ws = cb[:, 128:256]
            c_Wr = cb[:, 256:384]
            c_ablk = cb[:, 384:392]
            c_ones = cb[0:1, 392:904]
            c_brow = cb[0:1, 904:1032]
            c_iota16 = cb[:, 1032:1160]
            c_iotaC = c32[:, 128:129]

            use_inloop = os.environ.get("GAT_INLOOP", "0") == "1"
            rep_ctx = None
            if use_inloop:
                nrep_sb = cpool.tile([1, 1], mybir.dt.int32)
                nc.sync.dma_start(out=nrep_sb[:], in_=d_nrep[:])
                n_rep = nc.values_load(nrep_sb[0:1, 0:1], min_val=1,
                                       max_val=4096,
                                       skip_runtime_bounds_check=True)
                rep_ctx = tc.For_i(0, n_rep, 1)
                rep_ctx.__enter__()

            # ---- prologue: hr projection for local nodes ----
            for t in range(NTILES):
                pp = ps_lg.tile([P, 128], F32, tag="lg")
                nc.tensor.matmul(
                    out=pp[:], lhsT=nlt[:, t * P:(t + 1) * P],
                    rhs=c_Wr, start=True, stop=not add_bias,
                )
                if add_bias:
                    nc.tensor.matmul(
                        out=pp[:], lhsT=cb[0:1, 392:393], rhs=c_brow,
                        start=False, stop=True,
                    )
                dst = hr_sb[:, t * P:(t + 1) * P]
                if t % 2 == 0:
                    nc.scalar.activation(dst, pp[:], COPY)
                else:
                    nc.vector.tensor_copy(out=dst, in_=pp[:])

            # ---- main loop over receiver tiles ----
            for t in range(NTILES):
                co = t * T_max * P
                edg = sbio.tile([P, T_max * P], BF16, tag="edg")
                nc.sync.dma_start(out=edg[:], in_=d_edg[:, co:co + T_max * P])
                srt = sbio.tile([P, T_max * P], BF16, tag="srt")
                nc.sync.dma_start(out=srt[:], in_=d_srt[:, co:co + T_max * P])
                if use_bcast:
                    # receiver row replicated across partitions by the DMA
                    rrb = sbio.tile([P, T_max * P], BF16, tag="rrb")
                    nc.sync.dma_start(
                        out=rrb[:],
                        in_=d_rrow[0:1, co:co + T_max * P].to_broadcast(
                            [P, T_max * P]),
                    )
                    # whole-tile [node, edge] one-hot in one 4x-mode DVE op
                    ohne = sb.tile([P, T_max * P], BF16, tag="ohne")
                    nc.vector.tensor_scalar(out=ohne[:], in0=rrb[:],
                                            scalar1=c_iotaC, scalar2=None,
                                            op0=EQ)
                else:
                    rr = sbio.tile([1, T_max * P], BF16, tag="rr")
                    nc.sync.dma_start(out=rr[:],
                                      in_=d_rrow[:, co:co + T_max * P])

                acc = ps_acc.tile([P, 136], F32, tag="acc")
                hr_t = hr_sb[:, t * P:(t + 1) * P]
                n_sc = 0

                for gi, (gc0, ncg) in enumerate(GROUPS):
                    W = ncg * P
                    csl = slice(gc0 * P, gc0 * P + W)

                    if not use_bcast:
                        # broadcast the receiver row across partitions via PE
                        bc = ps_bc.tile([P, 512], F32, tag="bc")
                        nc.tensor.matmul(out=bc[:, 0:W],
                                         lhsT=c_ones[:, 0:128],
                                         rhs=rr[0:1, csl],
                                         start=True, stop=True)
                        ohne_g = sb.tile([P, GROUP * P], BF16, tag="ohne")
                        nc.vector.tensor_scalar(out=ohne_g[:, 0:W],
                                                in0=bc[:, 0:W],
                                                scalar1=c_iotaC, scalar2=None,
                                                op0=EQ)

                    zT = ps_zt.tile([P, 512], F32, tag="zT")
                    nc.tensor.matmul(out=zT[:, 0:W], lhsT=c_We, rhs=edg[:, csl],
                                     start=True, stop=False)
                    nc.tensor.matmul(out=zT[:, 0:W], lhsT=c_Ws, rhs=srt[:, csl],
                                     start=False, stop=False)
                    nc.tensor.matmul(out=zT[:, 0:W], lhsT=hr_t,
                                     rhs=ohne[:, csl] if use_bcast
                                     else ohne_g[:, 0:W],
                                     start=False, stop=not add_bias)
                    if add_bias:
                        nc.tensor.matmul(out=zT[:, 0:W], lhsT=c_brow,
                                         rhs=c_ones[:, 0:W],
                                         start=False, stop=True)

                    x = sb.tile([P, GROUP * P], BF16, tag="x")
                    nc.scalar.activation(x[:, 0:W], zT[:, 0:W], PRELU,
                                         alpha=0.01)

                    # logits directly in [edge, head] layout, per chunk
                    lg2 = ps_lg.tile([P, GROUP, 8], F32, tag="lg")
                    for c in range(ncg):
                        nc.tensor.matmul(
                            out=lg2[:, c, :],
                            lhsT=x[:, c * P:(c + 1) * P],
                            rhs=c_ablk, start=True, stop=True,
                        )
                    ex2 = sb.tile([P, GROUP, 8], F32, tag="ex2")
                    nc.scalar.activation(ex2[:, 0:ncg, :], lg2[:, 0:ncg, :],
                                         EXP)

                    # sent projection in [edge, dim] layout, per chunk
                    spj = ps_spj.tile([P, GROUP, 128], F32, tag="spj")
                    for c in range(ncg):
                        nc.tensor.matmul(
                            out=spj[:, c, :],
                            lhsT=srt[:, (gc0 + c) * P:(gc0 + c + 1) * P],
                            rhs=c_Ws, start=True, stop=True,
                        )

                    # scatter rhs: [weighted msg (128) | ex (8)] per chunk
                    rhs4 = sb.tile([P, GROUP, 136], BF16, tag="rhs4")
                    nc.scalar.activation(rhs4[:, 0:ncg, 128:136],
                                         ex2[:, 0:ncg, :], COPY)
                    nc.vector.tensor_tensor(
                        out=rhs4[:, 0:ncg, 0:128].rearrange(
                            "p c (h j) -> p c h j", h=8),
                        in0=spj[:, 0:ncg, :].rearrange(
                            "p c (h j) -> p c h j", h=8),
                        in1=ex2[:, 0:ncg, :].to_broadcast([P, ncg, 8, 16]),
                        op=MUL,
                    )

                    for c in range(ncg):
                        ohen = sb.tile([P, P], BF16, tag="ohen")
                        nc.vector.tensor_scalar(
                            out=ohen[:], in0=c_iota16,
                            scalar1=rloc[:, t * T_max + gc0 + c:
                                         t * T_max + gc0 + c + 1],
                            scalar2=None, op0=EQ,
                        )
                        n_sc += 1
                        nc.tensor.matmul(
                            out=acc[:],
                            lhsT=ohen[:],
                            rhs=rhs4[:, c, :],
                            start=(n_sc == 1), stop=(n_sc == T_max),
                        )

                # ---- epilogue ----
                dsb = sb.tile([P, 8], F32, tag="dsb")
                nc.vector.tensor_scalar(out=dsb[:], in0=acc[:, 128:136],
                                        scalar1=1e-30, scalar2=None, op0=ADD)
                rec = sb.tile([P, 8], F32, tag="rec")
                nc.vector.reciprocal(out=rec[:], in_=dsb[:])
                ot = sb.tile([P, P], F32, tag="ot")
                nc.vector.tensor_tensor(
                    out=ot[:].rearrange("p (h j) -> p h j", h=8),
                    in0=acc[:, 0:128].rearrange("p (h j) -> p h j", h=8),
                    in1=rec[:].to_broadcast([P, 8, 16]),
                    op=MUL,
                )
                nc.sync.dma_start(out=d_out[t * P:(t + 1) * P, :], in_=ot[:])

            if rep_ctx is not None:
                rep_ctx.__exit__(None, None, None)

    nc.compile()

    in_maps = [
        dict(EDG=EDG[ci], SRT=SRT[ci], RROW=RROW[ci], RLOC=RLOC[ci],
             NLT=NLT[ci], C16=C16, C32=C32,
             NREP=np.array([[int(os.environ.get("GAT_NREP", "1"))]],
                           dtype=np.int32))
        for ci in range(NCORES)
    ]

    def unpermute(ci, rows):
        out_c = np.zeros((NLOC, P), np.float32)
        valid = l2g[ci] >= 0
        out_c[l2g[ci][valid]] = rows[valid]
        return out_c

    if os.environ.get("GAT_SIM", "0") == "1":
        out0 = _run_sim(nc, in_maps[0])
        out = np.zeros((N, P), np.float32)
        out[:NLOC] = unpermute(0, out0)
        return out

    bench_iters = int(os.environ.get("GAT_BENCH", "0"))
    results = _run_pjrt(nc, in_maps, NCORES, bench_iters)
    out = np.concatenate(
        [unpermute(ci, results[ci]["OUT"]) for ci in range(NCORES)], axis=0
    )
    return out.astype(np.float32)


def _run_sim(nc, in_map):
    """Run core 0 through CoreSim: numerics check + cost-model timing."""
    from concourse.bass_interp import CoreSim, InstructionExecutor, Direction
    import concourse.mybir as mb

    class _Exec(InstructionExecutor):
        # CoreSim has no Prelu implementation; emulate it here (dev-only).
        def visit_InstActivation(self, instruction, *, reg_snapshot=None):
            if instruction.func != mb.ActivationFunctionType.Prelu:
                return super().visit_InstActivation(
                    instruction, reg_snapshot=reg_snapshot)
            alpha = instruction.ins[3]
            alpha = alpha.value if isinstance(alpha, mb.ImmediateValue) else 0.0
            iv = self.view_ap(instruction.ins[0], Direction.READ, instruction,
                              reg_snapshot=reg_snapshot).astype(np.float32)
            acted = np.where(iv > 0, iv, np.float32(alpha) * iv)
            ov = self.view_ap(instruction.outs[0], Direction.WRITE,
                              instruction, reg_snapshot=reg_snapshot)
            ov[:] = acted.astype(ov.dtype)

    sim = CoreSim(nc, publish_trace=False, executor_cls=_Exec)
    for name, arr in in_map.items():
        view = sim.tensor(name)
        view[:] = np.asarray(arr, dtype=view.dtype)
    sim.simulate()
    print(f"SIM time: {sim.time} ns")
    return np.array(sim.tensor("OUT"), dtype=np.float32)


def _run_pjrt(nc, in_maps, n_cores, bench_iters=0):
    """Execute the compiled module on the PJRT/axon devices; optionally
    re-run with pre-staged device inputs to measure steady-state latency."""
    global LAST_EXEC_NS, LAST_BENCH_NS
    import time as _time
    import jax
    from jax.sharding import Mesh, PartitionSpec, NamedSharding
    from jax.experimental.shard_map import shard_map
    import concourse.mybir as _mb
    from concourse import bass2jax as _b2j

    _b2j.install_neuronx_cc_hook()

    in_names, out_names, out_avals, zero_outs = [], [], [], []
    for alloc in nc.m.functions[0].allocations:
        if not isinstance(_mb.MemoryLocationSet, type) or not isinstance(alloc, _mb.MemoryLocationSet):
            continue
        name = alloc.memorylocations[0].name
        if alloc.kind == "ExternalInput":
            if nc.partition_id_tensor is None or name != nc.partition_id_tensor.name:
                in_names.append(name)
        elif alloc.kind == "ExternalOutput":
            out_names.append(name)
            shape = tuple(alloc.tensor_shape)
            dtype = _mb.dt.np(alloc.dtype)
            out_avals.append(jax.core.ShapedArray(shape, dtype))
            zero_outs.append(np.zeros(shape, dtype))
    n_params = len(in_names)
    n_outs = len(out_avals)
    in_names = in_names + out_names

    part_name = nc.partition_id_tensor.name if nc.partition_id_tensor else None
    if part_name is not None:
        in_names.append(part_name)

    def _body(*args):
        operands = list(args)
        if part_name is not None:
            operands.append(_b2j.partition_id_tensor())
        outs = _b2j._bass_exec_p.bind(
            *operands,
            out_avals=tuple(out_avals),
            in_names=tuple(in_names),
            out_names=tuple(out_names),
            lowering_input_output_aliases=(),
            sim_require_finite=True,
            sim_require_nnan=True,
            nc=nc,
        )
        return tuple(outs)

    devices = jax.devices()[:n_cores]
    mesh = Mesh(np.asarray(devices), ("core",))
    in_specs = (PartitionSpec("core"),) * (n_params + n_outs)
    out_specs = (PartitionSpec("core"),) * n_outs
    fn = jax.jit(
        shard_map(_body, mesh=mesh, in_specs=in_specs,
                  out_specs=out_specs, check_rep=False),
        keep_unused=True,
    )
    sh = NamedSharding(mesh, PartitionSpec("core"))
    concat_in = [
        jax.device_put(
            np.concatenate([np.asarray(in_maps[c][in_names[i]])
                            for c in range(n_cores)], axis=0), sh)
        for i in range(n_params)
    ]
    concat_zeros = [
        jax.device_put(np.zeros((n_cores * z.shape[0], *z.shape[1:]), z.dtype), sh)
        for z in zero_outs
    ]
    out_arrs = fn(*concat_in, *concat_zeros)
    jax.block_until_ready(out_arrs)

    if bench_iters > 0:
        use_inloop = os.environ.get("GAT_INLOOP", "0") == "1"
        if use_inloop:
            # Steady-state per-pass device time: the NEFF contains a dynamic
            # repeat loop over the whole kernel body (count = NREP input).
            # Time one execution at NREP=k1 and one at NREP=k2 and take the
            # slope: per-pass time measured entirely on-device, free of the
            # host<->terminal sync latency (~90ms) and per-dispatch overhead.
            nrep_idx = in_names.index("NREP")
            k1, k2 = 8, min(8 + 6 * bench_iters, 64)
            kprobe = os.environ.get("GAT_KPROBE", "")

            def run_point(k):
                arr = jax.device_put(
                    np.full((n_cores, 1), k, np.int32), sh)
                inputs = list(concat_in)
                inputs[nrep_idx] = arr
                t0 = _time.perf_counter()
                o = fn(*inputs, *concat_zeros)
                jax.block_until_ready(o)
                return _time.perf_counter() - t0
        else:
            # Amortized per-execution time: queue N executions back-to-back,
            # block once; the slope between two batch sizes removes the fixed
            # host<->terminal sync latency (~90ms on axon).
            k1, k2 = bench_iters, 3 * bench_iters

            def run_point(n):
                t0 = _time.perf_counter()
                outs = [fn(*concat_in, *concat_zeros) for _ in range(n)]
                jax.block_until_ready(outs)
                return _time.perf_counter() - t0

        if use_inloop and kprobe:
            for k in [int(s) for s in kprobe.split(",")]:
                t = run_point(k)
                print(f"KPROBE nrep={k}: {t*1e3:.2f} ms", flush=True)

        run_point(k1)  # warm
        run_point(k2)
        best = None
        for _ in range(4):
            t1 = run_point(k1)
            t2 = run_point(k2)
            slope = (t2 - t1) / (k2 - k1)
            if os.environ.get("GAT_VERBOSE", "0") == "1":
                print(f"pt {k1}: {t1*1e3:.2f} ms, pt {k2}: {t2*1e3:.2f} "
                      f"ms, slope {slope*1e3:.3f} ms/exec")
            if slope > 0 and (best is None or slope < best):
                best = slope
        if best is None:
            # all slopes were noise-negative; fall back to a conservative
            # whole-call-over-k2 estimate (includes host sync overhead)
            best = min(run_point(k2) for _ in range(2)) / k2
        LAST_BENCH_NS = int(best * 1e9)
        LAST_EXEC_NS = LAST_BENCH_NS

    np_outs = [np.asarray(a) for a in out_arrs]
    return [
        {name: np_outs[i].reshape(n_cores, *out_avals[i].shape)[c]
         for i, name in enumerate(out_names)}
        for c in range(n_cores)
    ]
